# revision 7
# baseline (speedup 1.0000x reference)
"""VQ codebook kernel for Trainium2 (8 NeuronCores, data-parallel over B).

Per core b: z = z_real[b] (4096, 256).
  scores t[i,j] = z_i . e_j - 0.5*||e_j||^2   (argmax_j t = argmin_j ||z_i - e_j||^2)
  computed on PE as an augmented K=257 matmul (2 K-tiles of z^T + rank-1 bias).
  z^T per row-tile via PE transpose (identity matmul).
  argmax via DVE InstMax + InstMaxIndex (exact fp32, first-index tie-break).
  codebook gather via gpsimd indirect DMA (emb rows from DRAM by idx).
  vq_loss identity: sum((zq-z)^2) = sum_i ||z_i||^2 - 2*sum_i max_i
    (so zq never needs to be in SBUF; ||z||^2 via ACT Square+accum).
z_imag passes through on the host.
"""

import os
import sys

import numpy as np

sys.path.insert(0, "/opt/trn_rl_repo")

B, L, D, NE = 8, 4096, 256, 1024
P = 128  # partitions
NT = L // P  # 32 row tiles per core
NB = NE // 512  # 2 PSUM banks of 512 scores

_compiled = {}
LAST_RESULT = None


def _build(mm_dtype_name="float32"):
    import concourse.bass as bass
    import concourse.mybir as mybir
    from concourse import bacc
    from concourse.masks import make_identity
    from concourse.tile import TileContext

    f32 = mybir.dt.float32
    mm_dt = getattr(mybir.dt, mm_dtype_name)

    nc = bacc.Bacc("TRN2", target_bir_lowering=False, debug=False)

    z_dram = nc.dram_tensor("z", [L, D], f32, kind="ExternalInput").ap()
    embT_dram = nc.dram_tensor("embT", [D, NE], f32, kind="ExternalInput").ap()
    ebias_dram = nc.dram_tensor("ebias", [1, NE], f32, kind="ExternalInput").ap()
    emb_dram = nc.dram_tensor("emb", [NE, D], f32, kind="ExternalInput").ap()
    zq_dram = nc.dram_tensor("zq", [L, D], f32, kind="ExternalOutput").ap()
    m_dram = nc.dram_tensor("mout", [P, NT], f32, kind="ExternalOutput").ap()
    xx_dram = nc.dram_tensor("xxout", [P, NT], f32, kind="ExternalOutput").ap()

    with TileContext(nc) as tc:
        with (
            tc.tile_pool(name="persist", bufs=1) as persist,
            tc.tile_pool(name="zn", bufs=3) as zn_pool,
            tc.tile_pool(name="zt", bufs=3) as zt_pool,
            tc.tile_pool(name="sc", bufs=3) as sc_pool,
            tc.tile_pool(name="small", bufs=4) as small,
            tc.tile_pool(name="ps_s", bufs=2, space="PSUM") as ps_s,
            tc.tile_pool(name="ps_t", bufs=2, space="PSUM") as ps_t,
        ):
            # --- codebook (transposed) + bias row + ones row + identity
            eT = []
            for k in range(2):
                t = persist.tile([P, NE], f32, tag=f"eT{k}")
                nc.sync.dma_start(out=t[:], in_=embT_dram[k * 128 : (k + 1) * 128, :])
                eT.append(t)
            ebias = persist.tile([1, NE], f32, tag="ebias")
            nc.sync.dma_start(out=ebias[:], in_=ebias_dram[:])
            ones = persist.tile([1, P], f32, tag="ones")
            nc.vector.memset(ones[:], 1.0)
            ident = persist.tile([P, P], f32, tag="ident")
            make_identity(nc, ident[:])

            # --- accumulators
            m_all = persist.tile([P, NT], f32, tag="m_all")
            idx_all = persist.tile([P, NT], mybir.dt.uint32, tag="idx_all")
            xx = persist.tile([P, NT], f32, tag="xx")
            zq_sb = persist.tile([P, NT * D], f32, tag="zq_sb")

            # --- main loop over row tiles
            for t in range(NT):
                zn = zn_pool.tile([P, D], f32, tag="zn")
                nc.sync.dma_start(out=zn[:], in_=z_dram[t * P : (t + 1) * P, :])

                # transpose z tile: zT[:, k*128:(k+1)*128] = (zn[:, kdims]).T
                zt = zt_pool.tile([P, D], f32, tag="zt")
                for k in range(2):
                    tp = ps_t.tile([P, P], f32)
                    nc.tensor.transpose(
                        tp[:], zn[:, k * P : (k + 1) * P], ident[:]
                    )
                    nc.vector.tensor_copy(zt[:, k * P : (k + 1) * P], tp[:])

                # row sums of z^2 (per-partition accum on ACT)
                sq = sc_pool.tile([P, D], f32, tag="sq")
                nc.scalar.activation(
                    sq[:],
                    zn[:],
                    mybir.ActivationFunctionType.Square,
                    accum_out=xx[:, t : t + 1],
                )

                s_ps = ps_s.tile([P, NE], f32)
                for b in range(NB):
                    cs = slice(b * 512, (b + 1) * 512)
                    for k in range(2):
                        nc.tensor.matmul(
                            s_ps[:, cs],
                            lhsT=zt[:, k * P : (k + 1) * P].bitcast(mm_dt),
                            rhs=eT[k][:, cs].bitcast(mm_dt),
                            start=(k == 0),
                            stop=False,
                        )
                    nc.tensor.matmul(
                        s_ps[:, cs],
                        lhsT=ones[:, :],
                        rhs=ebias[:, cs],
                        start=False,
                        stop=True,
                    )
                s_sb = sc_pool.tile([P, NE], f32, tag="s_sb")
                nc.scalar.activation(
                    s_sb[:], s_ps[:], mybir.ActivationFunctionType.Copy
                )
                max8 = small.tile([P, 8], f32, tag="max8")
                idx8 = small.tile([P, 8], mybir.dt.uint32, tag="idx8")
                nc.vector.max(out=max8[:], in_=s_sb[:])
                nc.vector.max_index(idx8[:], max8[:], s_sb[:])
                nc.vector.tensor_copy(m_all[:, t : t + 1], max8[:, 0:1])
                nc.vector.tensor_copy(idx_all[:, t : t + 1], idx8[:, 0:1])
                # gather this tile's codebook rows (one row per partition)
                nc.gpsimd.indirect_dma_start(
                    out=zq_sb[:, t * D : (t + 1) * D],
                    out_offset=None,
                    in_=emb_dram,
                    in_offset=bass.IndirectOffsetOnAxis(
                        ap=idx_all[:, t : t + 1], axis=0
                    ),
                )

            # --- outputs
            nc.sync.dma_start(
                out=zq_dram.rearrange("(g p) d -> p g d", p=P),
                in_=zq_sb[:].rearrange("p (g d) -> p g d", g=NT),
            )
            nc.sync.dma_start(out=m_dram, in_=m_all[:])
            nc.sync.dma_start(out=xx_dram, in_=xx[:])

    nc.finalize()
    return nc


def _get_nc():
    key = "v1"
    if key not in _compiled:
        _compiled[key] = _build()
    return _compiled[key]


def kernel(z_real, z_imag, embedding):
    from concourse.bass_utils import run_bass_kernel_spmd

    z_real = np.ascontiguousarray(z_real, dtype=np.float32)
    embedding = np.ascontiguousarray(embedding, dtype=np.float32)

    embT = np.ascontiguousarray(embedding.T)
    ebias = (-0.5 * (embedding.astype(np.float64) ** 2).sum(axis=1)).astype(
        np.float32
    )[None, :]

    nc = _get_nc()
    in_maps = [
        {
            "z": z_real[b],
            "embT": embT,
            "ebias": ebias,
            "emb": embedding,
        }
        for b in range(B)
    ]
    res = run_bass_kernel_spmd(nc, in_maps, list(range(B)))
    global LAST_RESULT
    LAST_RESULT = res

    zq = np.empty((B, L, D), dtype=np.float32)
    tot = 0.0  # sum over all of ||z||^2 - 2*m, in fp64
    for b in range(B):
        r = res.results[b]
        zq[b] = r["zq"]
        tot += float(r["xxout"].astype(np.float64).sum())
        tot -= 2.0 * float(r["mout"].astype(np.float64).sum())

    vq_loss = np.float32(1.25 * tot / (B * L * D))
    # straight-through estimator, replicated in fp32 exactly as the ref does
    zq_out_real = z_real + (zq - z_real)
    return zq_out_real, z_imag, vq_loss


# revision 10
# speedup vs baseline: 1.2421x; 1.2421x over previous
"""VQ codebook kernel for Trainium2 (8 NeuronCores, data-parallel over B).

Per core b: z = z_real[b] (4096, 256).
  scores t[i,j] = z_i . e_j - 0.5*||e_j||^2   (argmax_j t = argmin_j ||z_i - e_j||^2)
  computed on PE as an augmented K=257 matmul (2 K-tiles of z^T + rank-1 bias).
  z^T per row-tile via PE transpose (identity matmul).
  argmax via DVE InstMax + InstMaxIndex (exact fp32, first-index tie-break).
  codebook gather via gpsimd indirect DMA (emb rows from DRAM by idx).
  vq_loss identity: sum((zq-z)^2) = sum_i ||z_i||^2 - 2*sum_i max_i
    (so zq never needs to be in SBUF; ||z||^2 via ACT Square+accum).
z_imag passes through on the host.
"""

import os
import sys

import numpy as np

sys.path.insert(0, "/opt/trn_rl_repo")

B, L, D, NE = 8, 4096, 256, 1024
P = 128  # partitions
NT = L // P  # 32 row tiles per core
NB = NE // 512  # 2 PSUM banks of 512 scores

_compiled = {}
LAST_RESULT = None


def _build(mm_dtype_name="float32"):
    import concourse.bass as bass
    import concourse.mybir as mybir
    from concourse import bacc
    from concourse.masks import make_identity
    from concourse.tile import TileContext

    f32 = mybir.dt.float32
    mm_dt = getattr(mybir.dt, mm_dtype_name)

    nc = bacc.Bacc("TRN2", target_bir_lowering=False, debug=False)

    z_dram = nc.dram_tensor("z", [L, D], f32, kind="ExternalInput").ap()
    embT_dram = nc.dram_tensor("embT", [D, NE], f32, kind="ExternalInput").ap()
    ebias_h_dram = nc.dram_tensor("ebias_h", [1, NE], mybir.dt.float16, kind="ExternalInput").ap()
    ebias_l_dram = nc.dram_tensor("ebias_l", [1, NE], mybir.dt.float16, kind="ExternalInput").ap()
    emb_dram = nc.dram_tensor("emb", [NE, D], f32, kind="ExternalInput").ap()
    zq_dram = nc.dram_tensor("zq", [L, D], f32, kind="ExternalOutput").ap()
    m8_dram = nc.dram_tensor("m8out", [P, NT * 8], f32, kind="ExternalOutput").ap()
    i8_dram = nc.dram_tensor("i8out", [P, NT * 8], mybir.dt.uint32, kind="ExternalOutput").ap()
    xx_dram = nc.dram_tensor("xxout", [P, NT], f32, kind="ExternalOutput").ap()

    with TileContext(nc) as tc:
        with (
            tc.tile_pool(name="persist", bufs=1) as persist,
            tc.tile_pool(name="zn", bufs=3) as zn_pool,
            tc.tile_pool(name="zt", bufs=3) as zt_pool,
            tc.tile_pool(name="sc", bufs=3) as sc_pool,
            tc.tile_pool(name="small", bufs=4) as small,
            tc.tile_pool(name="ps_s", bufs=2, space="PSUM") as ps_s,
            tc.tile_pool(name="ps_t", bufs=2, space="PSUM") as ps_t,
        ):
            # --- codebook (transposed) + bias row + ones row + identity
            need_round = mm_dt != f32
            eT = []
            for k in range(2):
                t = persist.tile([P, NE], mm_dt, tag=f"eT{k}")
                if need_round:
                    raw = persist.tile([P, NE], f32, tag=f"eTraw{k}")
                    nc.sync.dma_start(
                        out=raw[:], in_=embT_dram[k * 128 : (k + 1) * 128, :]
                    )
                    nc.vector.tensor_copy(t[:], raw[:])
                else:
                    nc.sync.dma_start(
                        out=t[:], in_=embT_dram[k * 128 : (k + 1) * 128, :]
                    )
                eT.append(t)
            ebias_h = persist.tile([1, NE], mybir.dt.float16, tag="ebias_h")
            nc.sync.dma_start(out=ebias_h[:], in_=ebias_h_dram[:])
            ebias_l = persist.tile([1, NE], mybir.dt.float16, tag="ebias_l")
            nc.sync.dma_start(out=ebias_l[:], in_=ebias_l_dram[:])
            ones = persist.tile([1, P], mybir.dt.float16, tag="ones")
            nc.vector.memset(ones[:], 1.0)
            ident = persist.tile([P, P], f32, tag="ident")
            make_identity(nc, ident[:])

            # --- accumulators
            m_all = persist.tile([P, NT * 8], f32, tag="m_all")
            idx_all = persist.tile([P, NT * 8], mybir.dt.uint32, tag="idx_all")
            xx = persist.tile([P, NT], f32, tag="xx")
            zq_sb = persist.tile([P, NT * D], f32, tag="zq_sb")

            # --- main loop over row tiles
            for t in range(NT):
                zn = zn_pool.tile([P, D], f32, tag="zn")
                nc.sync.dma_start(out=zn[:], in_=z_dram[t * P : (t + 1) * P, :])

                # transpose z tile: zT[:, k*128:(k+1)*128] = (zn[:, kdims]).T
                zt = zt_pool.tile([P, D], mm_dt, tag="zt")
                for k in range(2):
                    tp = ps_t.tile([P, P], f32)
                    nc.tensor.transpose(
                        tp[:], zn[:, k * P : (k + 1) * P], ident[:]
                    )
                    nc.vector.tensor_copy(zt[:, k * P : (k + 1) * P], tp[:])

                # row sums of z^2 (per-partition accum on ACT)
                sq = sc_pool.tile([P, D], f32, tag="sq")
                nc.scalar.activation(
                    sq[:],
                    zn[:],
                    mybir.ActivationFunctionType.Square,
                    accum_out=xx[:, t : t + 1],
                )

                s_ps = ps_s.tile([P, NE], f32)
                for b in range(NB):
                    cs = slice(b * 512, (b + 1) * 512)
                    for k in range(2):
                        nc.tensor.matmul(
                            s_ps[:, cs],
                            lhsT=zt[:, k * P : (k + 1) * P],
                            rhs=eT[k][:, cs],
                            start=(k == 0),
                            stop=False,
                        )
                    nc.tensor.matmul(
                        s_ps[:, cs],
                        lhsT=ones[:, :],
                        rhs=ebias_h[:, cs],
                        start=False,
                        stop=False,
                    )
                    nc.tensor.matmul(
                        s_ps[:, cs],
                        lhsT=ones[:, :],
                        rhs=ebias_l[:, cs],
                        start=False,
                        stop=True,
                    )
                s_sb = sc_pool.tile([P, NE], f32, tag="s_sb")
                nc.scalar.activation(
                    s_sb[:], s_ps[:], mybir.ActivationFunctionType.Copy
                )
                max8 = small.tile([P, 8], f32, tag="max8")
                idx8 = small.tile([P, 8], mybir.dt.uint32, tag="idx8")
                nc.vector.max(out=max8[:], in_=s_sb[:])
                nc.vector.max_index(idx8[:], max8[:], s_sb[:])
                nc.vector.tensor_copy(m_all[:, t * 8 : (t + 1) * 8], max8[:])
                nc.vector.tensor_copy(idx_all[:, t * 8 : (t + 1) * 8], idx8[:])
                # gather this tile's codebook rows (one row per partition)
                nc.gpsimd.indirect_dma_start(
                    out=zq_sb[:, t * D : (t + 1) * D],
                    out_offset=None,
                    in_=emb_dram,
                    in_offset=bass.IndirectOffsetOnAxis(
                        ap=idx_all[:, t * 8 : t * 8 + 1], axis=0
                    ),
                )

            # --- outputs
            nc.sync.dma_start(
                out=zq_dram.rearrange("(g p) d -> p g d", p=P),
                in_=zq_sb[:].rearrange("p (g d) -> p g d", g=NT),
            )
            nc.sync.dma_start(out=m8_dram, in_=m_all[:])
            nc.sync.dma_start(out=i8_dram, in_=idx_all[:])
            nc.sync.dma_start(out=xx_dram, in_=xx[:])

    nc.finalize()
    return nc


def _get_nc():
    key = os.environ.get("MM_DTYPE", "float32")
    if key not in _compiled:
        _compiled[key] = _build(mm_dtype_name=key)
    return _compiled[key]


def kernel(z_real, z_imag, embedding):
    from concourse.bass_utils import run_bass_kernel_spmd

    z_real = np.ascontiguousarray(z_real, dtype=np.float32)
    embedding = np.ascontiguousarray(embedding, dtype=np.float32)

    embT = np.ascontiguousarray(embedding.T)
    ebias64 = -0.5 * (embedding.astype(np.float64) ** 2).sum(axis=1)
    ebias_h = ebias64.astype(np.float16)
    ebias_l = (ebias64 - ebias_h.astype(np.float64)).astype(np.float16)

    nc = _get_nc()
    in_maps = [
        {
            "z": z_real[b],
            "embT": embT,
            "ebias_h": ebias_h[None, :],
            "ebias_l": ebias_l[None, :],
            "emb": embedding,
        }
        for b in range(B)
    ]
    res = run_bass_kernel_spmd(nc, in_maps, list(range(B)))
    global LAST_RESULT
    LAST_RESULT = res

    MARGIN = 0.08  # rescore rows whose top-2 scores are this close
    e64 = embedding.astype(np.float64)
    eb64 = -0.5 * (e64**2).sum(axis=1)
    zq = np.empty((B, L, D), dtype=np.float32)
    tot = 0.0  # sum over all of ||z||^2 - 2*m, in fp64
    for b in range(B):
        r = res.results[b]
        zq[b] = r["zq"]
        tot += float(r["xxout"].astype(np.float64).sum())
        # m8/i8: [P, NT, 8]; row g*128+p -> [p, g]
        m8 = r["m8out"].reshape(P, NT, 8)
        i8 = r["i8out"].reshape(P, NT, 8)
        msum = m8[:, :, 0].astype(np.float64).sum()
        # exact rescoring of near-ties among the device's top-8
        amb = np.argwhere(m8[:, :, 0] - m8[:, :, 1] < MARGIN)
        if len(amb):
            zb = z_real[b].reshape(L, D).astype(np.float64)
            for p, g, in amb:
                row = g * P + p
                cand = i8[p, g].astype(np.int64)
                s = e64[cand] @ zb[row] + eb64[cand]
                kbest = int(np.argmax(s))
                best = int(cand[kbest])
                msum += s[kbest] - m8[p, g, 0]
                zq[b, row] = embedding[best]
        tot -= 2.0 * msum

    vq_loss = np.float32(1.25 * tot / (B * L * D))
    # straight-through estimator, replicated in fp32 exactly as the ref does
    zq_out_real = z_real + (zq - z_real)
    return zq_out_real, z_imag, vq_loss


# revision 11
# speedup vs baseline: 1.3023x; 1.0484x over previous
"""VQ codebook kernel for Trainium2 (8 NeuronCores, data-parallel over B).

Per core b: z = z_real[b] (4096, 256).
  scores t[i,j] = z_i . e_j - 0.5*||e_j||^2   (argmax_j t = argmin_j ||z_i - e_j||^2)
  computed on PE as an augmented K=257 matmul (2 K-tiles of z^T + rank-1 bias).
  z^T per row-tile via PE transpose (identity matmul).
  argmax via DVE InstMax + InstMaxIndex (exact fp32, first-index tie-break).
  codebook gather via gpsimd indirect DMA (emb rows from DRAM by idx).
  vq_loss identity: sum((zq-z)^2) = sum_i ||z_i||^2 - 2*sum_i max_i
    (so zq never needs to be in SBUF; ||z||^2 via ACT Square+accum).
z_imag passes through on the host.
"""

import os
import sys

import numpy as np

sys.path.insert(0, "/opt/trn_rl_repo")

B, L, D, NE = 8, 4096, 256, 1024
P = 128  # partitions
NT = L // P  # 32 row tiles per core
NB = NE // 512  # 2 PSUM banks of 512 scores

_compiled = {}
LAST_RESULT = None


def _build(mm_dtype_name="float32"):
    import concourse.bass as bass
    import concourse.mybir as mybir
    from concourse import bacc
    from concourse.masks import make_identity
    from concourse.tile import TileContext

    f32 = mybir.dt.float32
    mm_dt = getattr(mybir.dt, mm_dtype_name)

    nc = bacc.Bacc("TRN2", target_bir_lowering=False, debug=False)

    z_dram = nc.dram_tensor("z", [L, D], f32, kind="ExternalInput").ap()
    embT_dram = nc.dram_tensor("embT", [D, NE], f32, kind="ExternalInput").ap()
    ebias_h_dram = nc.dram_tensor("ebias_h", [1, NE], mybir.dt.float16, kind="ExternalInput").ap()
    ebias_l_dram = nc.dram_tensor("ebias_l", [1, NE], mybir.dt.float16, kind="ExternalInput").ap()
    emb_dram = nc.dram_tensor("emb", [NE, D], f32, kind="ExternalInput").ap()
    zq_dram = nc.dram_tensor("zq", [L, D], f32, kind="ExternalOutput").ap()
    m8_dram = nc.dram_tensor("m8out", [P, NT * 8], f32, kind="ExternalOutput").ap()
    i8_dram = nc.dram_tensor("i8out", [P, NT * 8], mybir.dt.uint32, kind="ExternalOutput").ap()
    xx_dram = nc.dram_tensor("xxout", [P, NT], f32, kind="ExternalOutput").ap()

    with TileContext(nc) as tc:
        with (
            tc.tile_pool(name="persist", bufs=1) as persist,
            tc.tile_pool(name="zn", bufs=3) as zn_pool,
            tc.tile_pool(name="zt", bufs=3) as zt_pool,
            tc.tile_pool(name="sc", bufs=3) as sc_pool,
            tc.tile_pool(name="small", bufs=4) as small,
            tc.tile_pool(name="ps_s", bufs=2, space="PSUM") as ps_s,
            tc.tile_pool(name="ps_t", bufs=2, space="PSUM") as ps_t,
        ):
            # --- codebook (transposed) + bias row + ones row + identity
            need_round = mm_dt != f32
            eT = []
            for k in range(2):
                t = persist.tile([P, NE], mm_dt, tag=f"eT{k}")
                if need_round:
                    raw = persist.tile([P, NE], f32, tag=f"eTraw{k}")
                    nc.sync.dma_start(
                        out=raw[:], in_=embT_dram[k * 128 : (k + 1) * 128, :]
                    )
                    nc.vector.tensor_copy(t[:], raw[:])
                else:
                    nc.sync.dma_start(
                        out=t[:], in_=embT_dram[k * 128 : (k + 1) * 128, :]
                    )
                eT.append(t)
            ebias_h = persist.tile([1, NE], mybir.dt.float16, tag="ebias_h")
            nc.sync.dma_start(out=ebias_h[:], in_=ebias_h_dram[:])
            ebias_l = persist.tile([1, NE], mybir.dt.float16, tag="ebias_l")
            nc.sync.dma_start(out=ebias_l[:], in_=ebias_l_dram[:])
            ones = persist.tile([1, P], mybir.dt.float16, tag="ones")
            nc.vector.memset(ones[:], 1.0)
            ident = persist.tile([P, P], f32, tag="ident")
            make_identity(nc, ident[:])

            # --- accumulators
            m_all = persist.tile([P, NT * 8], f32, tag="m_all")
            idx_all = persist.tile([P, NT * 8], mybir.dt.uint32, tag="idx_all")
            xx = persist.tile([P, NT], f32, tag="xx")
            zq_sb = persist.tile([P, NT * D], f32, tag="zq_sb")

            # --- main loop over row tiles
            for t in range(NT):
                zn = zn_pool.tile([P, D], f32, tag="zn")
                nc.sync.dma_start(out=zn[:], in_=z_dram[t * P : (t + 1) * P, :])

                # transpose z tile: zT[:, k*128:(k+1)*128] = (zn[:, kdims]).T
                zt = zt_pool.tile([P, D], mm_dt, tag="zt")
                for k in range(2):
                    tp = ps_t.tile([P, P], f32)
                    nc.tensor.transpose(
                        tp[:], zn[:, k * P : (k + 1) * P], ident[:]
                    )
                    nc.scalar.activation(
                        zt[:, k * P : (k + 1) * P],
                        tp[:],
                        mybir.ActivationFunctionType.Copy,
                    )

                # row sums of z^2 (per-partition accum on ACT)
                sq = sc_pool.tile([P, D], f32, tag="sq")
                nc.scalar.activation(
                    sq[:],
                    zn[:],
                    mybir.ActivationFunctionType.Square,
                    accum_out=xx[:, t : t + 1],
                )

                s_ps = ps_s.tile([P, NE], f32)
                for b in range(NB):
                    cs = slice(b * 512, (b + 1) * 512)
                    for k in range(2):
                        nc.tensor.matmul(
                            s_ps[:, cs],
                            lhsT=zt[:, k * P : (k + 1) * P],
                            rhs=eT[k][:, cs],
                            start=(k == 0),
                            stop=False,
                        )
                    nc.tensor.matmul(
                        s_ps[:, cs],
                        lhsT=ones[:, :],
                        rhs=ebias_h[:, cs],
                        start=False,
                        stop=False,
                    )
                    nc.tensor.matmul(
                        s_ps[:, cs],
                        lhsT=ones[:, :],
                        rhs=ebias_l[:, cs],
                        start=False,
                        stop=True,
                    )
                s_sb = sc_pool.tile([P, NE], f32, tag="s_sb")
                nc.scalar.activation(
                    s_sb[:], s_ps[:], mybir.ActivationFunctionType.Copy
                )
                max8 = small.tile([P, 8], f32, tag="max8")
                idx8 = small.tile([P, 8], mybir.dt.uint32, tag="idx8")
                nc.vector.max(out=max8[:], in_=s_sb[:])
                nc.vector.max_index(idx8[:], max8[:], s_sb[:])
                nc.vector.tensor_copy(m_all[:, t * 8 : (t + 1) * 8], max8[:])
                nc.vector.tensor_copy(idx_all[:, t * 8 : (t + 1) * 8], idx8[:])
                # gather this tile's codebook rows (one row per partition)
                nc.gpsimd.indirect_dma_start(
                    out=zq_sb[:, t * D : (t + 1) * D],
                    out_offset=None,
                    in_=emb_dram,
                    in_offset=bass.IndirectOffsetOnAxis(
                        ap=idx_all[:, t * 8 : t * 8 + 1], axis=0
                    ),
                )

            # --- outputs
            nc.sync.dma_start(
                out=zq_dram.rearrange("(g p) d -> p g d", p=P),
                in_=zq_sb[:].rearrange("p (g d) -> p g d", g=NT),
            )
            nc.sync.dma_start(out=m8_dram, in_=m_all[:])
            nc.sync.dma_start(out=i8_dram, in_=idx_all[:])
            nc.sync.dma_start(out=xx_dram, in_=xx[:])

    nc.finalize()
    return nc


def _get_nc():
    key = os.environ.get("MM_DTYPE", "float16")
    if key not in _compiled:
        _compiled[key] = _build(mm_dtype_name=key)
    return _compiled[key]


def kernel(z_real, z_imag, embedding):
    from concourse.bass_utils import run_bass_kernel_spmd

    z_real = np.ascontiguousarray(z_real, dtype=np.float32)
    embedding = np.ascontiguousarray(embedding, dtype=np.float32)

    embT = np.ascontiguousarray(embedding.T)
    ebias64 = -0.5 * (embedding.astype(np.float64) ** 2).sum(axis=1)
    ebias_h = ebias64.astype(np.float16)
    ebias_l = (ebias64 - ebias_h.astype(np.float64)).astype(np.float16)

    nc = _get_nc()
    in_maps = [
        {
            "z": z_real[b],
            "embT": embT,
            "ebias_h": ebias_h[None, :],
            "ebias_l": ebias_l[None, :],
            "emb": embedding,
        }
        for b in range(B)
    ]
    res = run_bass_kernel_spmd(nc, in_maps, list(range(B)))
    global LAST_RESULT
    LAST_RESULT = res

    mm_dtype = os.environ.get("MM_DTYPE", "float16")
    # rescore rows whose top-2 device scores are closer than the max plausible
    # device score error for the matmul dtype (x5+ safety factor)
    MARGIN = {"float32": 0.01, "float32r": 0.08, "float16": 0.35}[mm_dtype]
    e64 = embedding.astype(np.float64)
    eb64 = -0.5 * (e64**2).sum(axis=1)
    zq = np.empty((B, L, D), dtype=np.float32)
    tot = 0.0  # sum over all of ||z||^2 - 2*m, in fp64
    for b in range(B):
        r = res.results[b]
        zq[b] = r["zq"]
        tot += float(r["xxout"].astype(np.float64).sum())
        # m8/i8: [P, NT, 8]; row g*128+p -> [p, g]
        m8 = r["m8out"].reshape(P, NT, 8)
        i8 = r["i8out"].reshape(P, NT, 8)
        msum = m8[:, :, 0].astype(np.float64).sum()
        # exact rescoring of near-ties among the device's top-8
        amb = np.argwhere(m8[:, :, 0] - m8[:, :, 1] < MARGIN)
        if len(amb):
            zb = z_real[b].reshape(L, D).astype(np.float64)
            pp, gg = amb[:, 0], amb[:, 1]
            rows = gg * P + pp
            cand = i8[pp, gg].astype(np.int64)  # (n, 8)
            s = np.einsum("nd,nkd->nk", zb[rows], e64[cand]) + eb64[cand]
            kbest = np.argmax(s, axis=1)
            n = np.arange(len(rows))
            best = cand[n, kbest]
            msum += (s[n, kbest] - m8[pp, gg, 0].astype(np.float64)).sum()
            zq[b, rows] = embedding[best]
        tot -= 2.0 * msum

    vq_loss = np.float32(1.25 * tot / (B * L * D))
    # straight-through estimator, replicated in fp32 exactly as the ref does
    zq_out_real = z_real + (zq - z_real)
    return zq_out_real, z_imag, vq_loss


# revision 12
# speedup vs baseline: 1.7606x; 1.3519x over previous
"""VQ codebook kernel for Trainium2 (8 NeuronCores, data-parallel over B).

Per core b: z = z_real[b] (4096, 256).
  scores t[i,j] = z_i . e_j - 0.5*||e_j||^2   (argmax_j t = argmin_j ||z_i - e_j||^2)
  computed on PE in fp16 (z and codebook host-cast to fp16; z^T loaded via the
  16-bit DMA xbar transpose straight from DRAM; bias as a rank-1 fp16 matmul).
  argmax via DVE InstMax + InstMaxIndex (top-8 values+indices kept).
  codebook gather via per-tile gpsimd indirect DMA (fp32 emb rows from DRAM).
  vq_loss identity: sum((zq-z)^2) = sum_i ||z_i||^2 - 2*sum_i max_i
    (||z||^2 exactly in fp32 via ACT Square+accum on device).
Host post-pass: rows whose top-2 fp16 scores are within MARGIN are rescored
exactly (fp64) among the device's top-8 candidates, fixing any fp16-induced
argmax flips; this reproduces full-fp32 fidelity (reference-vs-fp64 flips only
occur below fp32 noise, far inside MARGIN).
z_imag passes through on the host.
"""

import os
import sys

import numpy as np

sys.path.insert(0, "/opt/trn_rl_repo")

B, L, D, NE = 8, 4096, 256, 1024
P = 128  # partitions
NT = L // P  # 32 row tiles per core
NB = NE // 512  # 2 PSUM banks of 512 scores

_compiled = {}
LAST_RESULT = None

# rescore rows whose top-2 device scores are closer than the max plausible
# device score error (worst-case fp16 product rounding ~0.1 + fp16 bias
# rounding ~0.06, x2+ safety)
MARGIN = 0.5


def _build():
    import concourse.bass as bass
    import concourse.mybir as mybir
    from concourse import bacc
    from concourse.tile import TileContext

    f32 = mybir.dt.float32
    f16 = mybir.dt.float16

    nc = bacc.Bacc("TRN2", target_bir_lowering=False, debug=False)

    z_dram = nc.dram_tensor("z", [L, D], f32, kind="ExternalInput").ap()
    z16_dram = nc.dram_tensor("z16", [L, D], f16, kind="ExternalInput").ap()
    embT_dram = nc.dram_tensor("embT16", [D, NE], f16, kind="ExternalInput").ap()
    ebias_dram = nc.dram_tensor("ebias16", [1, NE], f16, kind="ExternalInput").ap()
    emb_dram = nc.dram_tensor("emb", [NE, D], f32, kind="ExternalInput").ap()
    zq_dram = nc.dram_tensor("zq", [L, D], f32, kind="ExternalOutput").ap()
    m8_dram = nc.dram_tensor("m8out", [P, NT * 8], f32, kind="ExternalOutput").ap()
    i8_dram = nc.dram_tensor(
        "i8out", [P, NT * 8], mybir.dt.uint32, kind="ExternalOutput"
    ).ap()
    xx_dram = nc.dram_tensor("xxout", [P, NT], f32, kind="ExternalOutput").ap()

    with TileContext(nc) as tc:
        with (
            tc.tile_pool(name="persist", bufs=1) as persist,
            tc.tile_pool(name="zn", bufs=3) as zn_pool,
            tc.tile_pool(name="sc", bufs=3) as sc_pool,
            tc.tile_pool(name="small", bufs=4) as small,
            tc.tile_pool(name="ps_s", bufs=3, space="PSUM") as ps_s,
        ):
            # --- z^T via 16-bit xbar transpose: zt[k] [128, 4096] fp16
            zt = []
            for k in range(2):
                t = persist.tile([P, L], f16, tag=f"zt{k}")
                nc.sync.dma_start(
                    out=t[:],
                    in_=z16_dram[:, k * P : (k + 1) * P],
                    transpose=True,
                )
                zt.append(t)

            # --- codebook (transposed, fp16) + bias row + ones row
            eT = []
            for k in range(2):
                t = persist.tile([P, NE], f16, tag=f"eT{k}")
                nc.sync.dma_start(out=t[:], in_=embT_dram[k * P : (k + 1) * P, :])
                eT.append(t)
            ebias = persist.tile([1, NE], f16, tag="ebias")
            nc.sync.dma_start(out=ebias[:], in_=ebias_dram[:])
            ones = persist.tile([1, P], f16, tag="ones")
            nc.vector.memset(ones[:], 1.0)

            # --- accumulators
            m_all = persist.tile([P, NT * 8], f32, tag="m_all")
            idx_all = persist.tile([P, NT * 8], mybir.dt.uint32, tag="idx_all")
            xx = persist.tile([P, NT], f32, tag="xx")
            zq_sb = persist.tile([P, NT * D], f32, tag="zq_sb")

            # --- main loop over row tiles
            for t in range(NT):
                # exact row sums of z^2 (fp32) for the loss
                zn = zn_pool.tile([P, D], f32, tag="zn")
                nc.sync.dma_start(out=zn[:], in_=z_dram[t * P : (t + 1) * P, :])
                sq = sc_pool.tile([P, D], f32, tag="sq")
                nc.scalar.activation(
                    sq[:],
                    zn[:],
                    mybir.ActivationFunctionType.Square,
                    accum_out=xx[:, t : t + 1],
                )

                s_ps = ps_s.tile([P, NE], f32)
                for b in range(NB):
                    cs = slice(b * 512, (b + 1) * 512)
                    for k in range(2):
                        nc.tensor.matmul(
                            s_ps[:, cs],
                            lhsT=zt[k][:, t * P : (t + 1) * P],
                            rhs=eT[k][:, cs],
                            start=(k == 0),
                            stop=False,
                        )
                    nc.tensor.matmul(
                        s_ps[:, cs],
                        lhsT=ones[:, :],
                        rhs=ebias[:, cs],
                        start=False,
                        stop=True,
                    )
                s_sb = sc_pool.tile([P, NE], f32, tag="s_sb")
                nc.scalar.activation(
                    s_sb[:], s_ps[:], mybir.ActivationFunctionType.Copy
                )
                max8 = small.tile([P, 8], f32, tag="max8")
                idx8 = small.tile([P, 8], mybir.dt.uint32, tag="idx8")
                nc.vector.max(out=max8[:], in_=s_sb[:])
                nc.vector.max_index(idx8[:], max8[:], s_sb[:])
                nc.vector.tensor_copy(m_all[:, t * 8 : (t + 1) * 8], max8[:])
                nc.vector.tensor_copy(idx_all[:, t * 8 : (t + 1) * 8], idx8[:])
                # gather this tile's codebook rows (one row per partition)
                nc.gpsimd.indirect_dma_start(
                    out=zq_sb[:, t * D : (t + 1) * D],
                    out_offset=None,
                    in_=emb_dram,
                    in_offset=bass.IndirectOffsetOnAxis(
                        ap=idx_all[:, t * 8 : t * 8 + 1], axis=0
                    ),
                )

            # --- outputs
            nc.sync.dma_start(
                out=zq_dram.rearrange("(g p) d -> p g d", p=P),
                in_=zq_sb[:].rearrange("p (g d) -> p g d", g=NT),
            )
            nc.sync.dma_start(out=m8_dram, in_=m_all[:])
            nc.sync.dma_start(out=i8_dram, in_=idx_all[:])
            nc.sync.dma_start(out=xx_dram, in_=xx[:])

    nc.finalize()
    return nc


def _get_nc():
    key = "v2"
    if key not in _compiled:
        _compiled[key] = _build()
    return _compiled[key]


def kernel(z_real, z_imag, embedding):
    from concourse.bass_utils import run_bass_kernel_spmd

    z_real = np.ascontiguousarray(z_real, dtype=np.float32)
    embedding = np.ascontiguousarray(embedding, dtype=np.float32)

    z16 = z_real.astype(np.float16)
    embT16 = np.ascontiguousarray(embedding.T.astype(np.float16))
    ebias64 = -0.5 * (embedding.astype(np.float64) ** 2).sum(axis=1)
    ebias16 = ebias64.astype(np.float16)

    nc = _get_nc()
    in_maps = [
        {
            "z": z_real[b],
            "z16": z16[b],
            "embT16": embT16,
            "ebias16": ebias16[None, :],
            "emb": embedding,
        }
        for b in range(B)
    ]
    res = run_bass_kernel_spmd(nc, in_maps, list(range(B)))
    global LAST_RESULT
    LAST_RESULT = res

    e64 = embedding.astype(np.float64)
    eb64 = -0.5 * (e64**2).sum(axis=1)
    zq = np.empty((B, L, D), dtype=np.float32)
    tot = 0.0  # sum over all of ||z||^2 - 2*m, in fp64
    for b in range(B):
        r = res.results[b]
        zq[b] = r["zq"]
        tot += float(r["xxout"].astype(np.float64).sum())
        # m8/i8: [P, NT, 8]; row g*128+p -> [p, g]
        m8 = r["m8out"].reshape(P, NT, 8)
        i8 = r["i8out"].reshape(P, NT, 8)
        msum = m8[:, :, 0].astype(np.float64).sum()
        # exact rescoring of near-ties among the device's top-8
        amb = np.argwhere(m8[:, :, 0] - m8[:, :, 1] < MARGIN)
        if len(amb):
            zb = z_real[b].reshape(L, D).astype(np.float64)
            pp, gg = amb[:, 0], amb[:, 1]
            rows = gg * P + pp
            cand = i8[pp, gg].astype(np.int64)  # (n, 8)
            s = np.einsum("nd,nkd->nk", zb[rows], e64[cand]) + eb64[cand]
            kbest = np.argmax(s, axis=1)
            n = np.arange(len(rows))
            best = cand[n, kbest]
            msum += (s[n, kbest] - m8[pp, gg, 0].astype(np.float64)).sum()
            zq[b, rows] = embedding[best]
        tot -= 2.0 * msum

    vq_loss = np.float32(1.25 * tot / (B * L * D))
    # straight-through estimator, replicated in fp32 exactly as the ref does
    zq_out_real = z_real + (zq - z_real)
    return zq_out_real, z_imag, vq_loss


# revision 13
# speedup vs baseline: 1.9027x; 1.0807x over previous
"""VQ codebook kernel for Trainium2 (8 NeuronCores, data-parallel over B).

Per core b: z = z_real[b] (4096, 256).
  scores t[i,j] = z_i . e_j - 0.5*||e_j||^2   (argmax_j t = argmin_j ||z_i - e_j||^2)
  computed on PE in fp16 (z and codebook host-cast to fp16; z^T loaded via the
  16-bit DMA xbar transpose straight from DRAM; bias as a rank-1 fp16 matmul).
  argmax via DVE InstMax + InstMaxIndex (top-8 values+indices kept).
  codebook gather via per-tile gpsimd indirect DMA (fp32 emb rows from DRAM).
  vq_loss identity: sum((zq-z)^2) = sum_i ||z_i||^2 - 2*sum_i max_i
    (||z||^2 exactly in fp32 via ACT Square+accum on device).
Host post-pass: rows whose top-2 fp16 scores are within MARGIN are rescored
exactly (fp64) among the device's top-8 candidates, fixing any fp16-induced
argmax flips; this reproduces full-fp32 fidelity (reference-vs-fp64 flips only
occur below fp32 noise, far inside MARGIN).
z_imag passes through on the host.
"""

import os
import sys

import numpy as np

sys.path.insert(0, "/opt/trn_rl_repo")

B, L, D, NE = 8, 4096, 256, 1024
P = 128  # partitions
NT = L // P  # 32 row tiles per core
NB = NE // 512  # 2 PSUM banks of 512 scores

_compiled = {}
LAST_RESULT = None

# rescore rows whose top-2 device scores are closer than the max plausible
# device score error (worst-case fp16 product rounding ~0.1 + fp16 bias
# rounding ~0.06, x2+ safety)
MARGIN = 0.5


def _build():
    import concourse.bass as bass
    import concourse.mybir as mybir
    from concourse import bacc
    from concourse.tile import TileContext

    f32 = mybir.dt.float32
    f16 = mybir.dt.float16

    nc = bacc.Bacc("TRN2", target_bir_lowering=False, debug=False)

    z_dram = nc.dram_tensor("z", [L, D], f32, kind="ExternalInput").ap()
    z16_dram = nc.dram_tensor("z16", [L, D], f16, kind="ExternalInput").ap()
    embT_dram = nc.dram_tensor("embT16", [D, NE], f16, kind="ExternalInput").ap()
    ebias_dram = nc.dram_tensor("ebias16", [1, NE], f16, kind="ExternalInput").ap()
    emb_dram = nc.dram_tensor("emb", [NE, D], f32, kind="ExternalInput").ap()
    zq_dram = nc.dram_tensor("zq", [L, D], f32, kind="ExternalOutput").ap()
    m8_dram = nc.dram_tensor("m8out", [P, NT * 8], f32, kind="ExternalOutput").ap()
    i8_dram = nc.dram_tensor(
        "i8out", [P, NT * 8], mybir.dt.uint32, kind="ExternalOutput"
    ).ap()
    xx_dram = nc.dram_tensor("xxout", [P, NT], f32, kind="ExternalOutput").ap()

    with TileContext(nc) as tc:
        with (
            tc.tile_pool(name="persist", bufs=1) as persist,
            tc.tile_pool(name="zn", bufs=3) as zn_pool,
            tc.tile_pool(name="sc", bufs=3) as sc_pool,
            tc.tile_pool(name="small", bufs=4) as small,
            tc.tile_pool(name="ps_s", bufs=3, space="PSUM") as ps_s,
        ):
            # --- z^T via 16-bit xbar transpose: zt[k] [128, 4096] fp16
            zt = []
            for k in range(2):
                t = persist.tile([P, L], f16, tag=f"zt{k}")
                nc.sync.dma_start(
                    out=t[:],
                    in_=z16_dram[:, k * P : (k + 1) * P],
                    transpose=True,
                )
                zt.append(t)

            # --- codebook (transposed, fp16) + bias row + ones row
            eT = []
            for k in range(2):
                t = persist.tile([P, NE], f16, tag=f"eT{k}")
                nc.sync.dma_start(out=t[:], in_=embT_dram[k * P : (k + 1) * P, :])
                eT.append(t)
            ebias = persist.tile([1, NE], f16, tag="ebias")
            nc.sync.dma_start(out=ebias[:], in_=ebias_dram[:])
            ones = persist.tile([1, P], f16, tag="ones")
            nc.vector.memset(ones[:], 1.0)

            # --- accumulators
            m_all = persist.tile([P, NT * 8], f32, tag="m_all")
            idx_all = persist.tile([P, NT * 8], mybir.dt.uint32, tag="idx_all")
            xx = persist.tile([P, NT], f32, tag="xx")
            zq_sb = persist.tile([P, NT * D], f32, tag="zq_sb")

            # --- main loop over row tiles
            for t in range(NT):
                # exact row sums of z^2 (fp32) for the loss
                zn = zn_pool.tile([P, D], f32, tag="zn")
                nc.sync.dma_start(out=zn[:], in_=z_dram[t * P : (t + 1) * P, :])
                sq = sc_pool.tile([P, D], f32, tag="sq")
                nc.scalar.activation(
                    sq[:],
                    zn[:],
                    mybir.ActivationFunctionType.Square,
                    accum_out=xx[:, t : t + 1],
                )

                s_ps = ps_s.tile([P, NE], f32)
                # k-major order so consecutive matmuls share the stationary
                # operand (fewer weight reloads); bank accumulation groups
                # interleave via start/stop flags per PSUM region.
                for k in range(2):
                    for b in range(NB):
                        cs = slice(b * 512, (b + 1) * 512)
                        nc.tensor.matmul(
                            s_ps[:, cs],
                            lhsT=zt[k][:, t * P : (t + 1) * P],
                            rhs=eT[k][:, cs],
                            start=(k == 0),
                            stop=False,
                        )
                for b in range(NB):
                    cs = slice(b * 512, (b + 1) * 512)
                    nc.tensor.matmul(
                        s_ps[:, cs],
                        lhsT=ones[:, :],
                        rhs=ebias[:, cs],
                        start=False,
                        stop=True,
                    )
                s_sb = sc_pool.tile([P, NE], f32, tag="s_sb")
                nc.scalar.activation(
                    s_sb[:], s_ps[:], mybir.ActivationFunctionType.Copy
                )
                max8 = small.tile([P, 8], f32, tag="max8")
                idx8 = small.tile([P, 8], mybir.dt.uint32, tag="idx8")
                nc.vector.max(out=max8[:], in_=s_sb[:])
                nc.vector.max_index(idx8[:], max8[:], s_sb[:])
                nc.vector.tensor_copy(m_all[:, t * 8 : (t + 1) * 8], max8[:])
                nc.vector.tensor_copy(idx_all[:, t * 8 : (t + 1) * 8], idx8[:])
                # gather this tile's codebook rows (one row per partition)
                nc.gpsimd.indirect_dma_start(
                    out=zq_sb[:, t * D : (t + 1) * D],
                    out_offset=None,
                    in_=emb_dram,
                    in_offset=bass.IndirectOffsetOnAxis(
                        ap=idx_all[:, t * 8 : t * 8 + 1], axis=0
                    ),
                )

            # --- outputs
            nc.sync.dma_start(
                out=zq_dram.rearrange("(g p) d -> p g d", p=P),
                in_=zq_sb[:].rearrange("p (g d) -> p g d", g=NT),
            )
            nc.sync.dma_start(out=m8_dram, in_=m_all[:])
            nc.sync.dma_start(out=i8_dram, in_=idx_all[:])
            nc.sync.dma_start(out=xx_dram, in_=xx[:])

    nc.finalize()
    return nc


def _get_nc():
    key = "v2"
    if key not in _compiled:
        _compiled[key] = _build()
    return _compiled[key]


def kernel(z_real, z_imag, embedding):
    from concourse.bass_utils import run_bass_kernel_spmd

    z_real = np.ascontiguousarray(z_real, dtype=np.float32)
    embedding = np.ascontiguousarray(embedding, dtype=np.float32)

    z16 = z_real.astype(np.float16)
    embT16 = np.ascontiguousarray(embedding.T.astype(np.float16))
    ebias64 = -0.5 * (embedding.astype(np.float64) ** 2).sum(axis=1)
    ebias16 = ebias64.astype(np.float16)

    nc = _get_nc()
    in_maps = [
        {
            "z": z_real[b],
            "z16": z16[b],
            "embT16": embT16,
            "ebias16": ebias16[None, :],
            "emb": embedding,
        }
        for b in range(B)
    ]
    res = run_bass_kernel_spmd(nc, in_maps, list(range(B)))
    global LAST_RESULT
    LAST_RESULT = res

    e64 = embedding.astype(np.float64)
    eb64 = -0.5 * (e64**2).sum(axis=1)
    zq = np.empty((B, L, D), dtype=np.float32)
    tot = 0.0  # sum over all of ||z||^2 - 2*m, in fp64
    for b in range(B):
        r = res.results[b]
        zq[b] = r["zq"]
        tot += float(r["xxout"].astype(np.float64).sum())
        # m8/i8: [P, NT, 8]; row g*128+p -> [p, g]
        m8 = r["m8out"].reshape(P, NT, 8)
        i8 = r["i8out"].reshape(P, NT, 8)
        msum = m8[:, :, 0].astype(np.float64).sum()
        # exact rescoring of near-ties among the device's top-8
        amb = np.argwhere(m8[:, :, 0] - m8[:, :, 1] < MARGIN)
        if len(amb):
            zb = z_real[b].reshape(L, D).astype(np.float64)
            pp, gg = amb[:, 0], amb[:, 1]
            rows = gg * P + pp
            cand = i8[pp, gg].astype(np.int64)  # (n, 8)
            s = np.einsum("nd,nkd->nk", zb[rows], e64[cand]) + eb64[cand]
            kbest = np.argmax(s, axis=1)
            n = np.arange(len(rows))
            best = cand[n, kbest]
            msum += (s[n, kbest] - m8[pp, gg, 0].astype(np.float64)).sum()
            zq[b, rows] = embedding[best]
        tot -= 2.0 * msum

    vq_loss = np.float32(1.25 * tot / (B * L * D))
    # straight-through estimator, replicated in fp32 exactly as the ref does
    zq_out_real = z_real + (zq - z_real)
    return zq_out_real, z_imag, vq_loss


# revision 15
# speedup vs baseline: 1.9393x; 1.0193x over previous
"""VQ codebook kernel for Trainium2 (8 NeuronCores, data-parallel over B).

Per core b: z = z_real[b] (4096, 256).
  scores t[i,j] = z_i . e_j - 0.5*||e_j||^2   (argmax_j t = argmin_j ||z_i - e_j||^2)
  computed on PE in fp16 (z and codebook host-cast to fp16; z^T loaded via the
  16-bit DMA xbar transpose straight from DRAM; bias as a rank-1 fp16 matmul).
  argmax via DVE InstMax + InstMaxIndex (top-8 values+indices kept).
  codebook gather via per-tile gpsimd indirect DMA (fp32 emb rows from DRAM).
  vq_loss identity: sum((zq-z)^2) = sum_i ||z_i||^2 - 2*sum_i max_i
    (||z||^2 exactly in fp32 via ACT Square+accum on device).
Host post-pass: rows whose top-2 fp16 scores are within MARGIN are rescored
exactly (fp64) among the device's top-8 candidates, fixing any fp16-induced
argmax flips; this reproduces full-fp32 fidelity (reference-vs-fp64 flips only
occur below fp32 noise, far inside MARGIN).
z_imag passes through on the host.
"""

import os
import sys

import numpy as np

sys.path.insert(0, "/opt/trn_rl_repo")

B, L, D, NE = 8, 4096, 256, 1024
P = 128  # partitions
NT = L // P  # 32 row tiles per core
NB = NE // 512  # 2 PSUM banks of 512 scores

_compiled = {}
LAST_RESULT = None

# rescore rows whose top-2 device scores are closer than the max plausible
# device score error (worst-case fp16 product rounding ~0.1 + fp16 bias
# rounding ~0.06, x2+ safety)
MARGIN = 0.5


def _build():
    import concourse.bass as bass
    import concourse.mybir as mybir
    from concourse import bacc
    from concourse.tile import TileContext

    f32 = mybir.dt.float32
    f16 = mybir.dt.float16

    nc = bacc.Bacc("TRN2", target_bir_lowering=False, debug=False)

    z_dram = nc.dram_tensor("z", [L, D], f32, kind="ExternalInput").ap()
    z16_dram = nc.dram_tensor("z16", [L, D], f16, kind="ExternalInput").ap()
    embT_dram = nc.dram_tensor("embT16", [D, NE], f16, kind="ExternalInput").ap()
    ebias_dram = nc.dram_tensor("ebias16", [1, NE], f16, kind="ExternalInput").ap()
    emb_dram = nc.dram_tensor("emb", [NE, D], f32, kind="ExternalInput").ap()
    zq_dram = nc.dram_tensor("zq", [L, D], f32, kind="ExternalOutput").ap()
    m8_dram = nc.dram_tensor("m8out", [P, NT * 8], f32, kind="ExternalOutput").ap()
    i8_dram = nc.dram_tensor(
        "i8out", [P, NT * 8], mybir.dt.uint32, kind="ExternalOutput"
    ).ap()
    xx_dram = nc.dram_tensor("xxout", [P, NT], f32, kind="ExternalOutput").ap()

    with TileContext(nc) as tc:
        with (
            tc.tile_pool(name="persist", bufs=1) as persist,
            tc.tile_pool(name="zn", bufs=3) as zn_pool,
            tc.tile_pool(name="sc", bufs=3) as sc_pool,
            tc.tile_pool(name="small", bufs=4) as small,
            tc.tile_pool(name="ps_s", bufs=3, space="PSUM") as ps_s,
        ):
            # --- z^T via 16-bit xbar transpose: zt[k] [128, 4096] fp16
            zt = []
            for k in range(2):
                t = persist.tile([P, L], f16, tag=f"zt{k}")
                nc.sync.dma_start(
                    out=t[:],
                    in_=z16_dram[:, k * P : (k + 1) * P],
                    transpose=True,
                )
                zt.append(t)

            # --- codebook (transposed, fp16) + bias row + ones row
            eT = []
            for k in range(2):
                t = persist.tile([P, NE], f16, tag=f"eT{k}")
                nc.sync.dma_start(out=t[:], in_=embT_dram[k * P : (k + 1) * P, :])
                eT.append(t)
            ebias = persist.tile([1, NE], f16, tag="ebias")
            nc.sync.dma_start(out=ebias[:], in_=ebias_dram[:])
            ones = persist.tile([1, P], f16, tag="ones")
            nc.vector.memset(ones[:], 1.0)

            # --- accumulators
            m_all = persist.tile([P, NT * 8], f32, tag="m_all")
            idx_all = persist.tile([P, NT * 8], mybir.dt.uint32, tag="idx_all")
            xx = persist.tile([P, NT], f32, tag="xx")
            zq_sb = persist.tile([P, NT * D], f32, tag="zq_sb")

            # --- PE warmup burst (engage HAM 2.4 GHz before the real work)
            warm_ps = ps_s.tile([P, 512], f32, tag="s_ps")
            for _ in range(24):
                nc.tensor.matmul(
                    warm_ps[:], lhsT=eT[0][:, 0:P], rhs=eT[0][:, 0:512],
                    start=True, stop=True,
                )

            # --- main loop over row tiles
            for t in range(NT):
                # exact row sums of z^2 (fp32) for the loss
                zn = zn_pool.tile([P, D], f32, tag="zn")
                nc.sync.dma_start(out=zn[:], in_=z_dram[t * P : (t + 1) * P, :])
                sq = sc_pool.tile([P, D], f32, tag="sq")
                nc.scalar.activation(
                    sq[:],
                    zn[:],
                    mybir.ActivationFunctionType.Square,
                    accum_out=xx[:, t : t + 1],
                )

                s_ps = ps_s.tile([P, NE], f32)
                # k-major order so consecutive matmuls share the stationary
                # operand (fewer weight reloads); bank accumulation groups
                # interleave via start/stop flags per PSUM region.
                for k in range(2):
                    for b in range(NB):
                        cs = slice(b * 512, (b + 1) * 512)
                        nc.tensor.matmul(
                            s_ps[:, cs],
                            lhsT=zt[k][:, t * P : (t + 1) * P],
                            rhs=eT[k][:, cs],
                            start=(k == 0),
                            stop=False,
                        )
                for b in range(NB):
                    cs = slice(b * 512, (b + 1) * 512)
                    nc.tensor.matmul(
                        s_ps[:, cs],
                        lhsT=ones[:, :],
                        rhs=ebias[:, cs],
                        start=False,
                        stop=True,
                    )
                s_sb = sc_pool.tile([P, NE], f32, tag="s_sb")
                nc.scalar.activation(
                    s_sb[:], s_ps[:], mybir.ActivationFunctionType.Copy
                )
                max8 = m_all[:, t * 8 : (t + 1) * 8]
                idx8 = idx_all[:, t * 8 : (t + 1) * 8]
                nc.vector.max(out=max8, in_=s_sb[:])
                nc.vector.max_index(idx8, max8, s_sb[:])
                # gather this tile's codebook rows (one row per partition)
                nc.gpsimd.indirect_dma_start(
                    out=zq_sb[:, t * D : (t + 1) * D],
                    out_offset=None,
                    in_=emb_dram,
                    in_offset=bass.IndirectOffsetOnAxis(
                        ap=idx_all[:, t * 8 : t * 8 + 1], axis=0
                    ),
                )

            # --- outputs
            nc.sync.dma_start(
                out=zq_dram.rearrange("(g p) d -> p g d", p=P),
                in_=zq_sb[:].rearrange("p (g d) -> p g d", g=NT),
            )
            nc.sync.dma_start(out=m8_dram, in_=m_all[:])
            nc.sync.dma_start(out=i8_dram, in_=idx_all[:])
            nc.sync.dma_start(out=xx_dram, in_=xx[:])

    nc.finalize()
    return nc


def _get_nc():
    key = "v2"
    if key not in _compiled:
        _compiled[key] = _build()
    return _compiled[key]


def kernel(z_real, z_imag, embedding):
    from concourse.bass_utils import run_bass_kernel_spmd

    z_real = np.ascontiguousarray(z_real, dtype=np.float32)
    embedding = np.ascontiguousarray(embedding, dtype=np.float32)

    z16 = z_real.astype(np.float16)
    embT16 = np.ascontiguousarray(embedding.T.astype(np.float16))
    ebias64 = -0.5 * (embedding.astype(np.float64) ** 2).sum(axis=1)
    ebias16 = ebias64.astype(np.float16)

    nc = _get_nc()
    in_maps = [
        {
            "z": z_real[b],
            "z16": z16[b],
            "embT16": embT16,
            "ebias16": ebias16[None, :],
            "emb": embedding,
        }
        for b in range(B)
    ]
    res = run_bass_kernel_spmd(nc, in_maps, list(range(B)))
    global LAST_RESULT
    LAST_RESULT = res

    e64 = embedding.astype(np.float64)
    eb64 = -0.5 * (e64**2).sum(axis=1)
    zq = np.empty((B, L, D), dtype=np.float32)
    tot = 0.0  # sum over all of ||z||^2 - 2*m, in fp64
    for b in range(B):
        r = res.results[b]
        zq[b] = r["zq"]
        tot += float(r["xxout"].astype(np.float64).sum())
        # m8/i8: [P, NT, 8]; row g*128+p -> [p, g]
        m8 = r["m8out"].reshape(P, NT, 8)
        i8 = r["i8out"].reshape(P, NT, 8)
        msum = m8[:, :, 0].astype(np.float64).sum()
        # exact rescoring of near-ties among the device's top-8
        amb = np.argwhere(m8[:, :, 0] - m8[:, :, 1] < MARGIN)
        if len(amb):
            zb = z_real[b].reshape(L, D).astype(np.float64)
            pp, gg = amb[:, 0], amb[:, 1]
            rows = gg * P + pp
            cand = i8[pp, gg].astype(np.int64)  # (n, 8)
            s = np.einsum("nd,nkd->nk", zb[rows], e64[cand]) + eb64[cand]
            kbest = np.argmax(s, axis=1)
            n = np.arange(len(rows))
            best = cand[n, kbest]
            msum += (s[n, kbest] - m8[pp, gg, 0].astype(np.float64)).sum()
            zq[b, rows] = embedding[best]
        tot -= 2.0 * msum

    vq_loss = np.float32(1.25 * tot / (B * L * D))
    # straight-through estimator, replicated in fp32 exactly as the ref does
    zq_out_real = z_real + (zq - z_real)
    return zq_out_real, z_imag, vq_loss


# revision 16
# speedup vs baseline: 2.1328x; 1.0998x over previous
"""VQ codebook kernel for Trainium2 (8 NeuronCores, data-parallel over B).

Per core b: z = z_real[b] (4096, 256).
  scores t[i,j] = z_i . e_j - 0.5*||e_j||^2   (argmax_j t = argmin_j ||z_i - e_j||^2)
  computed on PE in fp16 (z and codebook host-cast to fp16; z^T loaded via the
  16-bit DMA xbar transpose straight from DRAM; bias as a rank-1 fp16 matmul).
  argmax via DVE InstMax + InstMaxIndex (top-8 values+indices kept).
  codebook gather via per-tile gpsimd indirect DMA (fp32 emb rows from DRAM).
  vq_loss identity: sum((zq-z)^2) = sum_i ||z_i||^2 - 2*sum_i max_i
    (||z||^2 exactly in fp32 via ACT Square+accum on device).
Host post-pass: rows whose top-2 fp16 scores are within MARGIN are rescored
exactly (fp64) among the device's top-8 candidates, fixing any fp16-induced
argmax flips; this reproduces full-fp32 fidelity (reference-vs-fp64 flips only
occur below fp32 noise, far inside MARGIN).
z_imag passes through on the host.
"""

import os
import sys

import numpy as np

sys.path.insert(0, "/opt/trn_rl_repo")

B, L, D, NE = 8, 4096, 256, 1024
P = 128  # partitions
NT = L // P  # 32 row tiles per core
NB = NE // 512  # 2 PSUM banks of 512 scores

_compiled = {}
LAST_RESULT = None

# rescore rows whose top-2 device scores are closer than the max plausible
# device score error (worst-case fp16 product rounding ~0.1 + fp16 bias
# rounding ~0.06, x2+ safety)
MARGIN = 0.5


def _build():
    import concourse.bass as bass
    import concourse.mybir as mybir
    from concourse import bacc
    from concourse.tile import TileContext

    f32 = mybir.dt.float32
    f16 = mybir.dt.float16

    nc = bacc.Bacc("TRN2", target_bir_lowering=False, debug=False)

    z_dram = nc.dram_tensor("z", [L, D], f32, kind="ExternalInput").ap()
    z16_dram = nc.dram_tensor("z16", [L, D], f16, kind="ExternalInput").ap()
    embT_dram = nc.dram_tensor("embT16", [D, NE], f16, kind="ExternalInput").ap()
    ebias_dram = nc.dram_tensor("ebias16", [1, NE], f16, kind="ExternalInput").ap()
    emb_dram = nc.dram_tensor("emb", [NE, D], f32, kind="ExternalInput").ap()
    zq_dram = nc.dram_tensor("zq", [L, D], f32, kind="ExternalOutput").ap()
    m8_dram = nc.dram_tensor("m8out", [P, NT * 8], f32, kind="ExternalOutput").ap()
    i8_dram = nc.dram_tensor(
        "i8out", [P, NT * 8], mybir.dt.uint32, kind="ExternalOutput"
    ).ap()
    xx_dram = nc.dram_tensor("xxout", [P, NT], f32, kind="ExternalOutput").ap()

    with TileContext(nc) as tc:
        with (
            tc.tile_pool(name="persist", bufs=1) as persist,
            tc.tile_pool(name="zn", bufs=4) as zn_pool,
            tc.tile_pool(name="sc", bufs=4) as sc_pool,
            tc.tile_pool(name="ps_s", bufs=4, space="PSUM") as ps_s,
        ):
            # --- z^T via 16-bit xbar transpose: zt[k] [128, 4096] fp16
            zt = []
            for k in range(2):
                t = persist.tile([P, L], f16, tag=f"zt{k}")
                for c in range(4):
                    rs = slice(c * (L // 4), (c + 1) * (L // 4))
                    nc.sync.dma_start(
                        out=t[:, rs],
                        in_=z16_dram[rs, k * P : (k + 1) * P],
                        transpose=True,
                    )
                zt.append(t)

            # --- codebook (transposed, fp16) + bias row + ones row
            eT = []
            for k in range(2):
                t = persist.tile([P, NE], f16, tag=f"eT{k}")
                nc.sync.dma_start(out=t[:], in_=embT_dram[k * P : (k + 1) * P, :])
                eT.append(t)
            ebias = persist.tile([1, NE], f16, tag="ebias")
            nc.sync.dma_start(out=ebias[:], in_=ebias_dram[:])
            ones = persist.tile([1, P], f16, tag="ones")
            nc.vector.memset(ones[:], 1.0)

            # --- accumulators
            m_all = persist.tile([P, NT * 8], f32, tag="m_all")
            idx_all = persist.tile([P, NT * 8], mybir.dt.uint32, tag="idx_all")
            xx = persist.tile([P, NT], f32, tag="xx")
            zq_sb = persist.tile([P, NT * D], f32, tag="zq_sb")

            # --- PE warmup burst (engage HAM 2.4 GHz before the real work)
            warm_ps = ps_s.tile([P, 512], f32, tag="s_ps")
            for _ in range(24):
                nc.tensor.matmul(
                    warm_ps[:], lhsT=eT[0][:, 0:P], rhs=eT[0][:, 0:512],
                    start=True, stop=True,
                )

            # --- main loop over row tiles
            for t in range(NT):
                # exact row sums of z^2 (fp32) for the loss
                zn = zn_pool.tile([P, D], f32, tag="zn")
                nc.sync.dma_start(out=zn[:], in_=z_dram[t * P : (t + 1) * P, :])
                sq = sc_pool.tile([P, D], f32, tag="sq")
                nc.scalar.activation(
                    sq[:],
                    zn[:],
                    mybir.ActivationFunctionType.Square,
                    accum_out=xx[:, t : t + 1],
                )

                s_ps = ps_s.tile([P, NE], f32)
                # k-major order so consecutive matmuls share the stationary
                # operand (fewer weight reloads); bank accumulation groups
                # interleave via start/stop flags per PSUM region.
                for k in range(2):
                    for b in range(NB):
                        cs = slice(b * 512, (b + 1) * 512)
                        nc.tensor.matmul(
                            s_ps[:, cs],
                            lhsT=zt[k][:, t * P : (t + 1) * P],
                            rhs=eT[k][:, cs],
                            start=(k == 0),
                            stop=False,
                        )
                for b in range(NB):
                    cs = slice(b * 512, (b + 1) * 512)
                    nc.tensor.matmul(
                        s_ps[:, cs],
                        lhsT=ones[:, :],
                        rhs=ebias[:, cs],
                        start=False,
                        stop=True,
                    )
                s_sb = sc_pool.tile([P, NE], f32, tag="s_sb")
                nc.scalar.activation(
                    s_sb[:], s_ps[:], mybir.ActivationFunctionType.Copy
                )
                max8 = m_all[:, t * 8 : (t + 1) * 8]
                idx8 = idx_all[:, t * 8 : (t + 1) * 8]
                nc.vector.max(out=max8, in_=s_sb[:])
                nc.vector.max_index(idx8, max8, s_sb[:])
                # gather this tile's codebook rows (one row per partition)
                nc.gpsimd.indirect_dma_start(
                    out=zq_sb[:, t * D : (t + 1) * D],
                    out_offset=None,
                    in_=emb_dram,
                    in_offset=bass.IndirectOffsetOnAxis(
                        ap=idx_all[:, t * 8 : t * 8 + 1], axis=0
                    ),
                )
                if t % 8 == 7:
                    gs = slice(t - 7, t + 1)
                    nc.sync.dma_start(
                        out=zq_dram.rearrange("(g p) d -> p g d", p=P)[:, gs, :],
                        in_=zq_sb[:].rearrange("p (g d) -> p g d", g=NT)[:, gs, :],
                    )

            # --- outputs
            nc.sync.dma_start(out=m8_dram, in_=m_all[:])
            nc.sync.dma_start(out=i8_dram, in_=idx_all[:])
            nc.sync.dma_start(out=xx_dram, in_=xx[:])

    nc.finalize()
    return nc


def _get_nc():
    key = "v2"
    if key not in _compiled:
        _compiled[key] = _build()
    return _compiled[key]


def kernel(z_real, z_imag, embedding):
    from concourse.bass_utils import run_bass_kernel_spmd

    z_real = np.ascontiguousarray(z_real, dtype=np.float32)
    embedding = np.ascontiguousarray(embedding, dtype=np.float32)

    z16 = z_real.astype(np.float16)
    embT16 = np.ascontiguousarray(embedding.T.astype(np.float16))
    ebias64 = -0.5 * (embedding.astype(np.float64) ** 2).sum(axis=1)
    ebias16 = ebias64.astype(np.float16)

    nc = _get_nc()
    in_maps = [
        {
            "z": z_real[b],
            "z16": z16[b],
            "embT16": embT16,
            "ebias16": ebias16[None, :],
            "emb": embedding,
        }
        for b in range(B)
    ]
    res = run_bass_kernel_spmd(nc, in_maps, list(range(B)))
    global LAST_RESULT
    LAST_RESULT = res

    e64 = embedding.astype(np.float64)
    eb64 = -0.5 * (e64**2).sum(axis=1)
    zq = np.empty((B, L, D), dtype=np.float32)
    tot = 0.0  # sum over all of ||z||^2 - 2*m, in fp64
    for b in range(B):
        r = res.results[b]
        zq[b] = r["zq"]
        tot += float(r["xxout"].astype(np.float64).sum())
        # m8/i8: [P, NT, 8]; row g*128+p -> [p, g]
        m8 = r["m8out"].reshape(P, NT, 8)
        i8 = r["i8out"].reshape(P, NT, 8)
        msum = m8[:, :, 0].astype(np.float64).sum()
        # exact rescoring of near-ties among the device's top-8
        amb = np.argwhere(m8[:, :, 0] - m8[:, :, 1] < MARGIN)
        if len(amb):
            zb = z_real[b].reshape(L, D).astype(np.float64)
            pp, gg = amb[:, 0], amb[:, 1]
            rows = gg * P + pp
            cand = i8[pp, gg].astype(np.int64)  # (n, 8)
            s = np.einsum("nd,nkd->nk", zb[rows], e64[cand]) + eb64[cand]
            kbest = np.argmax(s, axis=1)
            n = np.arange(len(rows))
            best = cand[n, kbest]
            msum += (s[n, kbest] - m8[pp, gg, 0].astype(np.float64)).sum()
            zq[b, rows] = embedding[best]
        tot -= 2.0 * msum

    vq_loss = np.float32(1.25 * tot / (B * L * D))
    # straight-through estimator, replicated in fp32 exactly as the ref does
    zq_out_real = z_real + (zq - z_real)
    return zq_out_real, z_imag, vq_loss
